# revision 80
# baseline (speedup 1.0000x reference)
"""Trainium2 Bass kernel for nn_CrossAttentionLayer (B=2,S=2048,H=768,NH=12).

Sharding: 8 cores = 2 batches x 4 head-groups (3 heads each, 192 cols).
Per core, everything runs in bf16 (inputs rounded on host) with fp32 PSUM
accumulation:
  - LN stats/normalize on DVE in [s,H] layout, then XBAR DMA-transpose the
    normalized tiles into xT [H,s] (no PE transposes).
  - q,k projections produce qT/kT [m,s]; v is projected directly into
    [s,m] layout with an interleaved ones-column per head (denominator).
  - scores s[k,q] = kT^T@qT per 128-key tile; exp on Act engine -> bf16.
  - attn-out av[q,m] = sum_t et[t]^T @ v[t]: t-major accumulation into 8
    parallel PSUM tiles, interleaved 2 tiles behind the exp stream so the
    Act engine (the critical resource at ~100us) never starves.
  - softmax division + dynamic_factor fold into one per-partition scale;
    scaled tiles are XBAR-transposed into cat [m,s]; output projection
    contracts 193 rows (192 m + dyn row for the bias) into PSUM, copied
    out via the (otherwise idle) Pool engine.
All non-exp/non-score work (projections, output proj) is software-pipelined
into the exp-stream gaps via a filler queue. Host sums the 4 partials per
batch (tensor-parallel unshard).
"""

import os
import sys
from collections import deque
from contextlib import ExitStack

import numpy as np

sys.path.insert(0, "/opt/trn_rl_repo")

import concourse.bass as bass
import concourse.bacc as bacc
import concourse.tile as tile
from concourse import mybir
from concourse.tile import TileContext

try:
    import ml_dtypes

    BF16_NP = ml_dtypes.bfloat16
except ImportError:  # pragma: no cover
    import jax.numpy as jnp

    BF16_NP = jnp.bfloat16

B, S, H, NH = 2, 2048, 768, 12
HD = H // NH            # 64
NG = 4                  # head groups
HL = H // NG            # 192 local cols (3 heads)
NHL = NH // NG          # 3 local heads
MEM_W = 0.5
LN_EPS = 1e-5

F32 = mybir.dt.float32
BF16 = mybir.dt.bfloat16

NT = S // 128           # 16 s-tiles
NC_ = 4                 # 512-wide chunks
CW = 1024               # query-chunk width in phase 3
NQB = CW // 128         # 8 query blocks per chunk
NCH = S // CW           # 2 chunks
MT = [(0, 128), (128, 64)]  # m-tiles of the 192 local cols
N_WARM = int(os.environ.get("N_WARM", "24"))  # PE p-state warmup matmuls

_CACHED = {}


def build_bass(debug=False):
    nc = bacc.Bacc()
    if debug:
        dbg_av = nc.declare_dram_parameter("dbg_av", [128, 1024], F32,
                                           isOutput=True)
        dbg_qT = nc.declare_dram_parameter("dbg_qT", [128, S], BF16,
                                           isOutput=True)
        dbg_kT = nc.declare_dram_parameter("dbg_kT", [128, S], BF16,
                                           isOutput=True)
        dbg_v = nc.declare_dram_parameter("dbg_v", [128, 195], BF16,
                                          isOutput=True)
        dbg_et = nc.declare_dram_parameter("dbg_et", [128, CW], BF16,
                                           isOutput=True)
        dbg_cat = nc.declare_dram_parameter("dbg_cat", [128, 2, S], BF16,
                                            isOutput=True)
        dbg_x = nc.declare_dram_parameter("dbg_x", [128, 6, S], BF16,
                                          isOutput=True)

    hid = nc.declare_dram_parameter("hid", [S, H], BF16, isOutput=False)
    crsT_d = nc.declare_dram_parameter("crsT", [H, S], BF16, isOutput=False)
    m0T_d = nc.declare_dram_parameter("m0T", [HL, S], BF16, isOutput=False)
    m1v = nc.declare_dram_parameter("m1v", [S, HL], BF16, isOutput=False)
    wq = nc.declare_dram_parameter("wq", [H, HL], BF16, isOutput=False)
    wk = nc.declare_dram_parameter("wk", [H, HL], BF16, isOutput=False)
    wv = nc.declare_dram_parameter("wv", [H, HL], BF16, isOutput=False)
    wo = nc.declare_dram_parameter("wo", [HL + 1, H], BF16, isOutput=False)
    bqv = nc.declare_dram_parameter("bqv", [128, 2], F32, isOutput=False)
    dynv = nc.declare_dram_parameter("dynv", [S], F32, isOutput=False)
    out = nc.declare_dram_parameter("out", [S, H], F32, isOutput=True)

    with TileContext(nc) as tc, ExitStack() as ctx:
        # ---- persistent pools ----
        singles = ctx.enter_context(tc.tile_pool(name="singles", bufs=1))
        qkp = ctx.enter_context(tc.tile_pool(name="qk", bufs=1))
        vp = ctx.enter_context(tc.tile_pool(name="vsb", bufs=1))
        catp = ctx.enter_context(tc.tile_pool(name="cat", bufs=1))

        wq_sb = singles.tile([128, 6, HL], BF16, name="wq_sb", tag="wq_sb")
        wk_sb = singles.tile([128, 6, HL], BF16, name="wk_sb", tag="wk_sb")
        wv_sb = singles.tile([128, 6, HL], BF16, name="wv_sb", tag="wv_sb")
        wo_sb0 = singles.tile([128, H], BF16, name="wo_sb0", tag="wo_sb0")
        wo_sb1 = singles.tile([65, H], BF16, name="wo_sb1", tag="wo_sb1")
        bq_sb = singles.tile([128, 2], F32)
        dyn_sb = singles.tile([128, NT], F32)
        dyn_bf = singles.tile([128, NT], BF16)

        # q/k transposed projections [m, s]
        qT = [qkp.tile([128, S], BF16, name="qT0", tag="qT0"),
              qkp.tile([64, S], BF16, name="qT1", tag="qT1")]
        kT = [qkp.tile([128, S], BF16, name="kT0", tag="kT0"),
              qkp.tile([64, S], BF16, name="kT1", tag="kT1")]
        # v in [s, m] layout with interleaved ones-columns: per head 65 cols
        v_sb = [vp.tile([128, 3 * 65], BF16, name=f"v{t}", tag=f"v{t}")
                for t in range(NT)]
        # cat [m, s] for the output projection, both halves in one tile so a
        # single XBAR transpose per s-tile fills it: plane 0 = m 0..127,
        # plane 1 rows 0..63 = m 128..191, row 64 = dyn (bias row), rows
        # 65..127 junk.
        catB = catp.tile([128, 2, S], BF16, tag="catB")

        with tc.tile_pool(name="hsp", bufs=1) as hsp, \
             tc.tile_pool(name="crsTp", bufs=1) as crsTp, \
             tc.tile_pool(name="xTp", bufs=1) as xTp, \
             tc.tile_pool(name="stats", bufs=4) as stats, \
             tc.tile_pool(name="mm512", bufs=2, space="PSUM") as mm512, \
             tc.tile_pool(name="sps", bufs=2, space="PSUM") as sps, \
             tc.tile_pool(name="avp", bufs=1, space="PSUM") as avp, \
             tc.tile_pool(name="etp", bufs=2) as etp, \
             tc.tile_pool(name="ap_", bufs=2) as ap_, \
             tc.tile_pool(name="srec", bufs=2) as srec:

            hs = hsp.tile([128, NT, H], BF16, name="hs", tag="hs")
            m0T0 = hsp.tile([128, S], BF16, name="m0T0", tag="m0T0")
            m0T1 = hsp.tile([64, S], BF16, name="m0T1", tag="m0T1")
            m1_sb = hsp.tile([128, NT, HL], BF16, name="m1_sb", tag="m1_sb")
            crsT = crsTp.tile([128, 6, S], BF16, name="crsT", tag="crsT")
            xT = xTp.tile([128, 6, S], BF16, name="xT", tag="xT")
            mv_all = stats.tile([128, 2, NT], F32, name="mv_all", tag="mv_all")
            rstd_all = stats.tile([128, NT], F32, name="rstd_all",
                                  tag="rstd_all")

            # ---- input DMAs, ordered for earliest exp start ----
            def dma_crsT(c):
                nc.sync.dma_start(
                    out=crsT[:, :, c * 512:(c + 1) * 512],
                    in_=crsT_d[:, c * 512:(c + 1) * 512].rearrange(
                        "(j p) s -> p j s", p=128))

            def dma_hs2(u):
                # 2-s-tile granularity so LN stats can start ~3us earlier
                nc.sync.dma_start(
                    out=hs[:, 2 * u:2 * (u + 1), :],
                    in_=hid[u * 256:(u + 1) * 256, :].rearrange(
                        "(c p) h -> p c h", p=128))

            # Early group: only what the first attention step needs. The
            # rest is emitted after phase A, giving it lower priority on the
            # contended DMA device than the latency-critical xT transposes.
            for u in range(4):
                dma_hs2(u)
            dma_crsT(0)
            nc.sync.dma_start(out=wk_sb,
                              in_=wk[:].rearrange("(j p) m -> p j m", p=128))
            nc.sync.dma_start(out=dyn_sb,
                              in_=dynv[:].rearrange("(c p) -> p c", p=128))
            nc.sync.dma_start(out=m0T0, in_=m0T_d[0:128, :])
            nc.sync.dma_start(out=wq_sb,
                              in_=wq[:].rearrange("(j p) m -> p j m", p=128))
            nc.sync.dma_start(out=bq_sb, in_=bqv[:])
            # Pool engine (SBUF-only work): keeps the DVE stream head clear
            # for the LN stats that gate the first exp
            nc.gpsimd.tensor_copy(dyn_bf, dyn_sb)
            for t in range(NT):
                for h in range(3):
                    nc.gpsimd.memset(v_sb[t][:, 65 * h + 64:65 * h + 65], 1.0)

            # ---- phase 1/2 emitters ----
            def emit_stats(st):
                t_ = hs[:, st, :]
                stt = stats.tile([128, 3, 6], F32, tag="st", name=f"st{st}")
                for sg in range(3):
                    nc.vector.bn_stats(out=stt[:, sg, :],
                                       in_=t_[:, sg * 256:(sg + 1) * 256])
                nc.vector.bn_aggr(out=mv_all[:, :, st], in_=stt)

            def emit_newton(c):
                # rstd = 1/sqrt(var+eps) on DVE via Newton (hidden_states is
                # ~N(0,1) so var+eps stays near 1 and y0=1 converges in 3
                # steps to ~1e-6) — keeps Sqrt (and its activation-table
                # load) off the Act engine, whose exp stream is the
                # critical resource.
                # runs on Pool: it is idle this early, so the 10-op serial
                # chain isn't stretched by greedy backfill the way it would
                # be between DVE stats ops
                sl = slice(4 * c, 4 * (c + 1))
                x = stats.tile([128, 4], F32, tag="nx", name=f"nx{c}")
                y = rstd_all[:, sl]
                nc.gpsimd.tensor_scalar(out=x, in0=mv_all[:, 1, sl],
                                        scalar1=LN_EPS, scalar2=None,
                                        op0=mybir.AluOpType.add)
                nc.gpsimd.tensor_scalar(out=y, in0=x, scalar1=-0.5,
                                        scalar2=1.5,
                                        op0=mybir.AluOpType.mult,
                                        op1=mybir.AluOpType.add)
                for it in range(2):
                    t2 = stats.tile([128, 4], F32, tag="nt", name=f"nt{c}{it}")
                    nc.gpsimd.tensor_tensor(t2, y, y, mybir.AluOpType.mult)
                    nc.gpsimd.tensor_tensor(t2, t2, x, mybir.AluOpType.mult)
                    nc.gpsimd.tensor_scalar(out=t2, in0=t2, scalar1=-0.5,
                                            scalar2=1.5,
                                            op0=mybir.AluOpType.mult,
                                            op1=mybir.AluOpType.add)
                    nc.gpsimd.tensor_tensor(y, y, t2, mybir.AluOpType.mult)

            def emit_norm(st):
                t_ = hs[:, st, :]
                nc.vector.tensor_scalar(out=t_, in0=t_,
                                        scalar1=mv_all[:, 0, st:st + 1],
                                        scalar2=rstd_all[:, st:st + 1],
                                        op0=mybir.AluOpType.subtract,
                                        op1=mybir.AluOpType.mult)
                nc.sync.dma_start_transpose(
                    xT[:, :, st * 128:(st + 1) * 128], t_)

            # Projection chains run in 256-wide chunks through a 4-deep pool
            # of 1-bank PSUM tiles, and their PSUM drains go to the Pool
            # engine — DVE is saturated with LN stats exactly when these
            # need to retire, and the pool rotation would otherwise chain
            # every projection to a stalled DVE copy.
            def emit_kproj(c, mi):
                m0_, msz = MT[mi]
                ps = mm512.tile([128, 512], F32, tag="mm", name=f"psk{c}{mi}")
                for j in range(6):
                    nc.tensor.matmul(ps[:msz], wk_sb[:, j, m0_:m0_ + msz],
                                     crsT[:, j, c * 512:(c + 1) * 512],
                                     start=(j == 0), stop=(j == 5))
                src = (m0T0[:, c * 512:(c + 1) * 512] if mi == 0
                       else m0T1[:, c * 512:(c + 1) * 512])
                nc.vector.tensor_tensor(kT[mi][:, c * 512:(c + 1) * 512],
                                        ps[:msz], src, mybir.AluOpType.add)

            def emit_qproj(c, mi):
                m0_, msz = MT[mi]
                ps = mm512.tile([128, 512], F32, tag="mm", name=f"psq{c}{mi}")
                for j in range(6):
                    nc.tensor.matmul(ps[:msz], wq_sb[:, j, m0_:m0_ + msz],
                                     xT[:, j, c * 512:(c + 1) * 512],
                                     start=(j == 0), stop=(j == 5))
                nc.vector.tensor_scalar(
                    out=qT[mi][:, c * 512:(c + 1) * 512],
                    in0=ps[:msz], scalar1=bq_sb[:msz, mi:mi + 1],
                    scalar2=None, op0=mybir.AluOpType.add)

            def emit_vproj(st):
                ps = mm512.tile([128, 512], F32, tag="mm", name=f"psv{st}")
                pv = ps[:, 0:HL]
                for j in range(6):
                    nc.tensor.matmul(pv, crsT[:, j, st * 128:(st + 1) * 128],
                                     wv_sb[:, j, :],
                                     start=(j == 0), stop=(j == 5))
                nc.vector.tensor_tensor(
                    v_sb[st].rearrange("p (h m) -> p h m", m=65)[:, :, 0:64],
                    pv.rearrange("p (h m) -> p h m", m=64),
                    m1_sb[:, st, :].rearrange("p (h m) -> p h m", m=64),
                    mybir.AluOpType.add)

            # ---- phase A: ONLY the first-exp critical chain at high
            # priority. The Tile scheduler is greedy by (ready, emission
            # priority), so everything emitted later still hoists into idle
            # gaps automatically — emission position is a deadline, not a
            # start time.
            for st in range(4):
                emit_stats(st)
            emit_newton(0)
            for st in range(4):
                emit_norm(st)
            for st in range(4, 8):
                emit_stats(st)
            emit_newton(1)
            for st in range(4, 8):
                emit_norm(st)
            # crsT chunks 1-3: must be emitted before the kproj consumers
            # below; half-chunk granularity so a bulk transfer never blocks
            # a just-became-ready xT transpose for long.
            def dma_crsT_half(c, v):
                nc.sync.dma_start(
                    out=crsT[:, :, c * 512 + v * 256:c * 512 + (v + 1) * 256],
                    in_=crsT_d[:, c * 512 + v * 256:c * 512 + (v + 1) * 256]
                    .rearrange("(j p) s -> p j s", p=128))

            for c in (1, 2, 3):
                dma_crsT_half(c, 0)
                dma_crsT_half(c, 1)

            # creation order drives the PSUM pool rotation: kproj(0,0) is
            # ready first, so it must own the first buffer. All m-tile-0
            # k projections fit in the pre-exp PE idle window, so no score
            # ever waits on kT.
            emit_kproj(0, 0)
            emit_qproj(0, 0)
            emit_qproj(1, 0)
            # kproj(1..3, 0) moved into step-0's filler queue: emitted after
            # the first score+exp, so the exp's batched PE-sem tick does not
            # cover them (they retire late, gated on the crsT chunk DMAs,
            # and would otherwise hold the first exp hostage).

            # Late input group: lower priority than everything above.
            nc.sync.dma_start(out=wv_sb,
                              in_=wv[:].rearrange("(j p) m -> p j m", p=128))
            nc.sync.dma_start(
                out=m1_sb, in_=m1v[:].rearrange("(c p) m -> p c m", p=128))
            for u in range(4, 8):
                dma_hs2(u)
            nc.sync.dma_start(out=m0T1, in_=m0T_d[128:192, :])
            nc.sync.dma_start(out=wo_sb0, in_=wo[0:128, :])
            nc.sync.dma_start(out=wo_sb1, in_=wo[128:193, :])

            # ---- phase 3: attention steps; bulk work is emitted at its
            # deadline position inside the exp-paced loop.
            steps = [(n, h) for n in range(NCH) for h in range(3)]

            def emit_ln_tail(c):
                for st in range(4 * c, 4 * (c + 1)):
                    emit_stats(st)
                emit_newton(c)
                for st in range(4 * c, 4 * (c + 1)):
                    emit_norm(st)

            fillers = deque()
            late_fillers = deque()
            step_fillers = {
                0: [lambda: emit_kproj(1, 0), lambda: emit_kproj(2, 0),
                    lambda: emit_kproj(3, 0)] +
                   [lambda t=t: emit_vproj(t) for t in range(NT)],
                1: [lambda: emit_kproj(0, 1), lambda: emit_kproj(1, 1),
                    lambda: emit_kproj(2, 1), lambda: emit_kproj(3, 1),
                    lambda: emit_qproj(0, 1), lambda: emit_qproj(1, 1),
                    lambda: emit_ln_tail(2)],
                2: [lambda: emit_ln_tail(3),
                    lambda: emit_qproj(2, 0), lambda: emit_qproj(3, 0),
                    lambda: emit_qproj(2, 1), lambda: emit_qproj(3, 1)],
                3: [], 4: [], 5: [],
            }

            a_tiles = {}
            # 8 attn-out accumulators packed into one 2-bank PSUM tile; qb=7
            # starts at the second bank so no slice straddles a boundary.
            av_big = avp.tile([128, 1024], F32, tag="av", name="av_big")
            av_tiles = [av_big[:, qb * 65:qb * 65 + 65] if qb < 7
                        else av_big[:, 512:577] for qb in range(NQB)]


            def head_rows(h, tens):
                return tens[0][64 * h:64 * h + 64, :] if h < 2 else \
                    tens[1][0:64, :]

            def emit_av_zero():
                # The 8 packed accumulators share PSUM zero-regions, so
                # matmul start=True zeroing is poison (each start re-marks
                # the whole 2KB region pending-zero, wiping its neighbours'
                # partial sums). Zero explicitly and accumulate-only.
                nc.vector.memset(av_big[:, 0:7 * 65], 0.0)
                nc.vector.memset(av_big[:, 512:577], 0.0)

            def emit_attnv_t(k_idx, t, qb0=0, qb1=NQB):
                n, h = steps[k_idx]
                et = get_et(k_idx, t)
                for qb in range(qb0, qb1):
                    nc.tensor.matmul(av_tiles[qb],
                                     et[:, qb * 128:(qb + 1) * 128],
                                     v_sb[t][:, 65 * h:65 * h + 65],
                                     start=False, stop=(t == NT - 1),
                                     skip_group_check=True)

            def emit_scale_qb(k_idx, qb):
                n, h = steps[k_idx]
                st = n * NQB + qb
                if debug and k_idx == 0 and qb == 0:
                    dav = srec.tile([128, 512], F32, tag="dav", name="dav",
                                    bufs=1)
                    nc.vector.tensor_copy(dav[:, 0:455], av_big[:, 0:455])
                    nc.sync.dma_start(out=dbg_av[:, 0:455], in_=dav[:, 0:455])
                    nc.vector.tensor_copy(dav[:, 0:65], av_big[:, 512:577])
                    nc.sync.dma_start(out=dbg_av[:, 512:577], in_=dav[:, 0:65])
                if (n, qb) not in a_tiles:
                    a_tiles[(n, qb)] = ap_.tile([128, 256], BF16,
                                                tag=f"a{qb}", bufs=2,
                                                name=f"a{qb}_{n}")
                at = a_tiles[(n, qb)]
                av = av_tiles[qb]
                r = srec.tile([128, 1], F32, tag="r", name=f"r{k_idx}{qb}")
                nc.vector.reciprocal(out=r, in_=av[:, 64:65])
                nc.vector.tensor_tensor(r, r, dyn_sb[:, st:st + 1],
                                        mybir.AluOpType.mult)
                nc.vector.tensor_scalar(out=at[:, 64 * h:64 * h + 64],
                                        in0=av[:, 0:64], scalar1=r,
                                        scalar2=None,
                                        op0=mybir.AluOpType.mult)
                if h == 2:
                    nc.vector.tensor_copy(
                        at[:, 192:256],
                        dyn_bf[:, st:st + 1].to_broadcast((128, 64)))
                    nc.sync.dma_start_transpose(
                        catB[:, :, st * 128:(st + 1) * 128], at[:])
                    del a_tiles[(n, qb)]

            def emit_scales(k_idx):
                for qb in range(NQB):
                    emit_scale_qb(k_idx, qb)
                # re-zero for the next step's accumulate-only attn-v
                emit_av_zero()

            def emit_outproj(st):
                ot = srec.tile([128, H], F32, tag="ot", bufs=6,
                               name=f"ot{st}")
                for hi, n0 in enumerate((0, 384)):
                    wp = mm512.tile([128, 512], F32, tag="mm",
                                    name=f"wp{st}_{n0}")
                    nc.tensor.matmul(wp[:, 0:384],
                                     catB[:, 0, st * 128:(st + 1) * 128],
                                     wo_sb0[:, n0:n0 + 384],
                                     start=True, stop=False)
                    nc.tensor.matmul(wp[:, 0:384],
                                     catB[0:65, 1, st * 128:(st + 1) * 128],
                                     wo_sb1[:, n0:n0 + 384],
                                     start=False, stop=True)
                    # chunk-0 copies go Pool-only: DVE must stay clear for
                    # the softmax scales (an ot-copy stuck in the DVE stream
                    # head-of-line-blocks them and stalls the whole cat/
                    # outproj pipeline). The drain chunk has no scales left,
                    # so it splits across both engines.
                    if st < NQB + 4 or hi == 0:
                        nc.vector.tensor_copy(ot[:, n0:n0 + 384], wp[:, 0:384])
                    else:
                        # drain-chunk second halves on the (by then idle) Act
                        nc.scalar.activation(
                            out=ot[:, n0:n0 + 384], in_=wp[:, 0:384],
                            func=mybir.ActivationFunctionType.Copy, bias=0.0)
                nc.sync.dma_start(
                    out=out[st * 128:(st + 1) * 128, :], in_=ot)

            et_tiles = {}

            def get_et(k, t):
                d = et_tiles.setdefault(k, {})
                if t not in d:
                    d[t] = etp.tile([128, CW], BF16, tag=f"e{t}", bufs=1,
                                    name=f"e{t}_{k}")
                return d[t]

            def emit_pretile(nk):
                # Next step's first score tile, computed into the (idle at
                # step tail) mm512 pool + exp'd ahead of the boundary: the
                # Act stream rolls straight into step nk without waiting
                # for the sps buffer rotation to free a score tile.
                nn2, nh2 = steps[nk]
                kk2 = head_rows(nh2, kT)
                qq2 = head_rows(nh2, qT)
                w = 512 if nk == len(steps) - 1 else CW
                for v2 in range(w // 512):
                    psb = mm512.tile([128, 512], F32, tag="mm",
                                     name=f"pre{nk}{v2}")
                    c0 = nn2 * CW + v2 * 512
                    nc.tensor.matmul(psb, kk2[:, 0:128], qq2[:, c0:c0 + 512],
                                     start=True, stop=True)
                    nc.scalar.activation(
                        out=get_et(nk, 0)[:, v2 * 512:(v2 + 1) * 512],
                        in_=psb,
                        func=mybir.ActivationFunctionType.Exp, scale=1.0)

            for k_idx, (n, h) in enumerate(steps):
                kk = head_rows(h, kT)
                qq = head_rows(h, qT)
                fillers.extend(step_fillers[k_idx])
                if k_idx == 3:
                    late_fillers.extend(
                        [lambda st=st: emit_outproj(st) for st in range(0, 4)])
                elif k_idx == 4:
                    late_fillers.extend(
                        [lambda st=st: emit_outproj(st) for st in range(4, NQB)])
                for t in range(NT):
                    get_et(k_idx, t)
                last = k_idx == len(steps) - 1
                first = k_idx == 0
                if first:
                    emit_av_zero()
                # The final step runs as two 512-wide half-chunks so the
                # first half's softmax scales / cat transpose / output
                # projection overlap the second half's exps instead of all
                # landing in the post-Act drain.
                halves = ((0, 512), (1, 512)) if last else ((0, CW),)
                for u, uw in halves:
                    for t in range(NT):
                        sp = sps.tile([128, CW], F32, tag="sc",
                                      name=f"sp{k_idx}{u}{t}")
                        # the very first score tile runs as two 512-wide
                        # halves so the first exp gates only on qproj(0,0),
                        # not on the whole first q-chunk
                        splitx = first and t == 0
                        for v2 in range(uw // 512):
                            c0 = n * CW + u * 512 + v2 * 512
                            nc.tensor.matmul(
                                sp[:, v2 * 512:(v2 + 1) * 512],
                                kk[:, t * 128:(t + 1) * 128],
                                qq[:, c0:c0 + 512],
                                start=True, stop=True)
                            if splitx:
                                nc.scalar.activation(
                                    out=get_et(k_idx, t)[
                                        :, v2 * 512:(v2 + 1) * 512],
                                    in_=sp[:, v2 * 512:(v2 + 1) * 512],
                                    func=mybir.ActivationFunctionType.Exp,
                                    scale=1.0)
                        if not splitx:
                            nc.scalar.activation(
                                out=get_et(k_idx, t)[:, u * 512:u * 512 + uw],
                                in_=sp[:, 0:uw],
                                func=mybir.ActivationFunctionType.Exp,
                                scale=1.0)
                        if debug and k_idx == 0 and t == 0 and u == 0:
                            nc.sync.dma_start(out=dbg_et[:],
                                              in_=get_et(0, 0)[:])
                        # drain filler emissions fast enough that producers
                        # (e.g. v projections) are always emitted before
                        # their consumers; the scheduler floats them into
                        # whatever idle slots exist.
                        for _ in range(2 if len(fillers) > 10 else 1):
                            if fillers:
                                fillers.popleft()()
                        if t >= 8 and late_fillers:
                            late_fillers.popleft()()
                        if last:
                            # u0's softmax scales interleave into u1's loop:
                            # emitting them between the halves would give
                            # them earlier per-engine sem ticks than u1's
                            # scores, and the monotone tick waits would then
                            # serialize u1's whole stream behind the u0
                            # scale/cat chain (~5us Act gap).
                            if u == 1 and t < 8 and t % 2 == 0:
                                emit_scale_qb(k_idx, t // 2)
                            if t >= 8:
                                emit_attnv_t(k_idx, t - 8, u * 4, u * 4 + 4)
                        elif t >= 7:
                            # emission lag keeps these from
                            # head-of-line-blocking the next scores while
                            # the previous step's scales still own av
                            emit_attnv_t(k_idx, t - 7)
                    if last:
                        for t_ in range(NT - 8, NT):
                            emit_attnv_t(k_idx, t_, u * 4, u * 4 + 4)
                        if u == 0:
                            late_fillers.extend(
                                [lambda st=st: emit_outproj(st)
                                 for st in range(n * NQB, n * NQB + 4)])
                        else:
                            for qb in range(4, 8):
                                emit_scale_qb(k_idx, qb)
                            for st in range(n * NQB + 4, n * NQB + 8):
                                emit_outproj(st)
                if not last:
                    for t in range(NT - 7, NT):
                        emit_attnv_t(k_idx, t)
                    def s2(k, q0):
                        emit_scale_qb(k, q0)
                        emit_scale_qb(k, q0 + 1)
                        if q0 == 6:
                            # re-zero for the next step's accumulate-only
                            # attn-v; popped well before its lag-7 emission
                            emit_av_zero()
                    for q0 in (0, 2, 4, 6):
                        fillers.append(lambda k=k_idx, q=q0: s2(k, q))

            # ---- drain ----
            while fillers:
                fillers.popleft()()
            while late_fillers:
                late_fillers.popleft()()

            if debug:
                nc.sync.dma_start(out=dbg_qT[:], in_=qT[0][:])
                nc.sync.dma_start(out=dbg_kT[:], in_=kT[0][:])
                nc.sync.dma_start(out=dbg_v[:], in_=v_sb[0][:])
                nc.sync.dma_start(out=dbg_cat[:], in_=catB[:])
                nc.sync.dma_start(out=dbg_x[:], in_=xT[:])

    nc.compile()
    return nc


def make_in_maps(inputs):
    bf = lambda a: np.asarray(np.asarray(a, np.float32), BF16_NP)
    hs = np.asarray(inputs["hidden_states"], np.float32)
    cs = np.asarray(inputs["cross_states"], np.float32)
    mem = np.asarray(inputs["memory_tensors"], np.float32)
    dyn = np.asarray(inputs["dynamic_factor"], np.float32)
    Wq = np.asarray(inputs["Wq"], np.float32)
    Wk = np.asarray(inputs["Wk"], np.float32)
    Wv = np.asarray(inputs["Wv"], np.float32)
    Wo = np.asarray(inputs["Wo"], np.float32)
    bq = np.asarray(inputs["bq"], np.float32)
    bv = np.asarray(inputs["bv"], np.float32)
    bo = np.asarray(inputs["bo"], np.float32)
    gate = float(np.asarray(inputs["gate"]).reshape(-1)[0])
    gate_bias = float(np.asarray(inputs["gate_bias"]).reshape(-1)[0])
    ln_g = np.asarray(inputs["ln_g"], np.float32)
    ln_b = np.asarray(inputs["ln_b"], np.float32)

    isq = 1.0 / np.sqrt(HD)
    in_maps = []
    for core in range(8):
        b, g = divmod(core, NG)
        cols = slice(g * HL, (g + 1) * HL)
        wq_eff = ln_g[:, None] * Wq[:, cols] * isq
        bq_eff = (bq[cols] + ln_b @ Wq[:, cols]) * isq
        bq_pack = np.zeros((128, 2), np.float32)
        bq_pack[:, 0] = bq_eff[0:128]
        bq_pack[:64, 1] = bq_eff[128:192]
        wo_ext = np.zeros((HL + 1, H), np.float32)
        wo_ext[:HL] = Wo[cols, :] * gate
        if g == 0:
            wo_ext[HL] = bo * gate + gate_bias
        in_maps.append({
            "hid": bf(hs[b]),
            "crsT": bf(np.ascontiguousarray(cs[b].T)),
            "m0T": bf(np.ascontiguousarray((mem[0, b][:, cols] * MEM_W).T)),
            "m1v": bf(mem[1, b][:, cols] * MEM_W + bv[cols]),
            "wq": bf(wq_eff),
            "wk": bf(Wk[:, cols]),
            "wv": bf(Wv[:, cols]),
            "wo": bf(wo_ext),
            "bqv": np.ascontiguousarray(bq_pack),
            "dynv": np.ascontiguousarray(dyn[b, :, 0]),
        })
    return in_maps


def kernel(**inputs):
    mask = np.asarray(inputs["attention_mask"])
    if not np.all(mask != 0):
        raise NotImplementedError("kernel specialized for all-ones attention_mask")

    if "nc" not in _CACHED:
        _CACHED["nc"] = build_bass()
    nc = _CACHED["nc"]

    from concourse.bass_utils import run_bass_kernel_spmd
    in_maps = make_in_maps(inputs)
    trace = bool(int(os.environ.get("KERNEL_TRACE", "0")))
    r = run_bass_kernel_spmd(nc, in_maps, list(range(8)), trace=trace)
    _CACHED["exec_time_ns"] = r.exec_time_ns
    _CACHED["profile_json"] = r.profile_json
    _CACHED["trace"] = r.instructions_and_trace
    res = r.results

    out = np.zeros((B, S, H), np.float32)
    for core in range(8):
        b = core // NG
        out[b] += res[core]["out"]
    return out


# revision 84
# speedup vs baseline: 1.0072x; 1.0072x over previous
"""Trainium2 Bass kernel for nn_CrossAttentionLayer (B=2,S=2048,H=768,NH=12).

Sharding: 8 cores = 2 batches x 4 head-groups (3 heads each, 192 cols).
Per core, everything runs in bf16 (inputs rounded on host) with fp32 PSUM
accumulation:
  - LN stats/normalize on DVE in [s,H] layout, then XBAR DMA-transpose the
    normalized tiles into xT [H,s] (no PE transposes).
  - q,k projections produce qT/kT [m,s]; v is projected directly into
    [s,m] layout with an interleaved ones-column per head (denominator).
  - scores s[k,q] = kT^T@qT per 128-key tile; exp on Act engine -> bf16.
  - attn-out av[q,m] = sum_t et[t]^T @ v[t]: t-major accumulation into 8
    parallel PSUM tiles, interleaved 2 tiles behind the exp stream so the
    Act engine (the critical resource at ~100us) never starves.
  - softmax division + dynamic_factor fold into one per-partition scale;
    scaled tiles are XBAR-transposed into cat [m,s]; output projection
    contracts 193 rows (192 m + dyn row for the bias) into PSUM, copied
    out via the (otherwise idle) Pool engine.
All non-exp/non-score work (projections, output proj) is software-pipelined
into the exp-stream gaps via a filler queue. Host sums the 4 partials per
batch (tensor-parallel unshard).
"""

import os
import sys
from collections import deque
from contextlib import ExitStack

import numpy as np

sys.path.insert(0, "/opt/trn_rl_repo")

import concourse.bass as bass
import concourse.bacc as bacc
import concourse.tile as tile
from concourse import mybir
from concourse.tile import TileContext

try:
    import ml_dtypes

    BF16_NP = ml_dtypes.bfloat16
except ImportError:  # pragma: no cover
    import jax.numpy as jnp

    BF16_NP = jnp.bfloat16

B, S, H, NH = 2, 2048, 768, 12
HD = H // NH            # 64
NG = 4                  # head groups
HL = H // NG            # 192 local cols (3 heads)
NHL = NH // NG          # 3 local heads
MEM_W = 0.5
LN_EPS = 1e-5

F32 = mybir.dt.float32
BF16 = mybir.dt.bfloat16

NT = S // 128           # 16 s-tiles
NC_ = 4                 # 512-wide chunks
CW = 1024               # query-chunk width in phase 3
NQB = CW // 128         # 8 query blocks per chunk
NCH = S // CW           # 2 chunks
MT = [(0, 128), (128, 64)]  # m-tiles of the 192 local cols
N_WARM = int(os.environ.get("N_WARM", "24"))  # PE p-state warmup matmuls

_CACHED = {}


def build_bass(debug=False):
    nc = bacc.Bacc()
    if debug:
        dbg_av = nc.declare_dram_parameter("dbg_av", [128, 1024], F32,
                                           isOutput=True)
        dbg_qT = nc.declare_dram_parameter("dbg_qT", [128, S], BF16,
                                           isOutput=True)
        dbg_kT = nc.declare_dram_parameter("dbg_kT", [128, S], BF16,
                                           isOutput=True)
        dbg_v = nc.declare_dram_parameter("dbg_v", [128, 195], BF16,
                                          isOutput=True)
        dbg_et = nc.declare_dram_parameter("dbg_et", [128, CW], BF16,
                                           isOutput=True)
        dbg_cat = nc.declare_dram_parameter("dbg_cat", [128, 2, S], BF16,
                                            isOutput=True)
        dbg_x = nc.declare_dram_parameter("dbg_x", [128, 6, S], BF16,
                                          isOutput=True)

    hid = nc.declare_dram_parameter("hid", [S, H], BF16, isOutput=False)
    crsT_d = nc.declare_dram_parameter("crsT", [H, S], BF16, isOutput=False)
    m0T_d = nc.declare_dram_parameter("m0T", [HL, S], BF16, isOutput=False)
    m1v = nc.declare_dram_parameter("m1v", [S, HL], BF16, isOutput=False)
    wq = nc.declare_dram_parameter("wq", [H, HL], BF16, isOutput=False)
    wk = nc.declare_dram_parameter("wk", [H, HL], BF16, isOutput=False)
    wv = nc.declare_dram_parameter("wv", [H, HL], BF16, isOutput=False)
    wo = nc.declare_dram_parameter("wo", [HL + 1, H], BF16, isOutput=False)
    bqv = nc.declare_dram_parameter("bqv", [128, 2], F32, isOutput=False)
    dynv = nc.declare_dram_parameter("dynv", [S], F32, isOutput=False)
    out = nc.declare_dram_parameter("out", [S, H], F32, isOutput=True)

    with TileContext(nc) as tc, ExitStack() as ctx:
        # ---- persistent pools ----
        singles = ctx.enter_context(tc.tile_pool(name="singles", bufs=1))
        qkp = ctx.enter_context(tc.tile_pool(name="qk", bufs=1))
        vp = ctx.enter_context(tc.tile_pool(name="vsb", bufs=1))
        catp = ctx.enter_context(tc.tile_pool(name="cat", bufs=1))

        wq_sb = singles.tile([128, 6, HL], BF16, name="wq_sb", tag="wq_sb")
        wk_sb = singles.tile([128, 6, HL], BF16, name="wk_sb", tag="wk_sb")
        wv_sb = singles.tile([128, 6, HL], BF16, name="wv_sb", tag="wv_sb")
        wo_sb0 = singles.tile([128, H], BF16, name="wo_sb0", tag="wo_sb0")
        wo_sb1 = singles.tile([65, H], BF16, name="wo_sb1", tag="wo_sb1")
        bq_sb = singles.tile([128, 2], F32)
        dyn_sb = singles.tile([128, NT], F32)
        dyn_bf = singles.tile([128, NT], BF16)

        # q/k transposed projections [m, s]
        qT = [qkp.tile([128, S], BF16, name="qT0", tag="qT0"),
              qkp.tile([64, S], BF16, name="qT1", tag="qT1")]
        kT = [qkp.tile([128, S], BF16, name="kT0", tag="kT0"),
              qkp.tile([64, S], BF16, name="kT1", tag="kT1")]
        # v in [s, m] layout with interleaved ones-columns: per head 65 cols
        v_sb = [vp.tile([128, 3 * 65], BF16, name=f"v{t}", tag=f"v{t}")
                for t in range(NT)]
        # cat [m, s] for the output projection, both halves in one tile so a
        # single XBAR transpose per s-tile fills it: plane 0 = m 0..127,
        # plane 1 rows 0..63 = m 128..191, row 64 = dyn (bias row), rows
        # 65..127 junk.
        catB = catp.tile([128, 2, S], BF16, tag="catB")

        with tc.tile_pool(name="hsp", bufs=1) as hsp, \
             tc.tile_pool(name="crsTp", bufs=1) as crsTp, \
             tc.tile_pool(name="xTp", bufs=1) as xTp, \
             tc.tile_pool(name="stats", bufs=4) as stats, \
             tc.tile_pool(name="mm512", bufs=2, space="PSUM") as mm512, \
             tc.tile_pool(name="sps", bufs=2, space="PSUM") as sps, \
             tc.tile_pool(name="avp", bufs=1, space="PSUM") as avp, \
             tc.tile_pool(name="etp", bufs=2) as etp, \
             tc.tile_pool(name="ap_", bufs=2) as ap_, \
             tc.tile_pool(name="srec", bufs=2) as srec:

            hs = hsp.tile([128, NT, H], BF16, name="hs", tag="hs")
            m0T0 = hsp.tile([128, S], BF16, name="m0T0", tag="m0T0")
            m0T1 = hsp.tile([64, S], BF16, name="m0T1", tag="m0T1")
            m1_sb = hsp.tile([128, NT, HL], BF16, name="m1_sb", tag="m1_sb")
            crsT = crsTp.tile([128, 6, S], BF16, name="crsT", tag="crsT")
            xT = xTp.tile([128, 6, S], BF16, name="xT", tag="xT")
            mv_all = stats.tile([128, 2, NT], F32, name="mv_all", tag="mv_all")
            rstd_all = stats.tile([128, NT], F32, name="rstd_all",
                                  tag="rstd_all")

            # ---- input DMAs, ordered for earliest exp start ----
            def dma_crsT(c):
                nc.sync.dma_start(
                    out=crsT[:, :, c * 512:(c + 1) * 512],
                    in_=crsT_d[:, c * 512:(c + 1) * 512].rearrange(
                        "(j p) s -> p j s", p=128))

            def dma_hs2(u):
                # 2-s-tile granularity so LN stats can start ~3us earlier
                nc.sync.dma_start(
                    out=hs[:, 2 * u:2 * (u + 1), :],
                    in_=hid[u * 256:(u + 1) * 256, :].rearrange(
                        "(c p) h -> p c h", p=128))

            # Early group: only what the first attention step needs. The
            # rest is emitted after phase A, giving it lower priority on the
            # contended DMA device than the latency-critical xT transposes.
            for u in range(4):
                dma_hs2(u)
            dma_crsT(0)
            nc.sync.dma_start(out=wk_sb,
                              in_=wk[:].rearrange("(j p) m -> p j m", p=128))
            nc.sync.dma_start(out=dyn_sb,
                              in_=dynv[:].rearrange("(c p) -> p c", p=128))
            nc.sync.dma_start(out=m0T0, in_=m0T_d[0:128, :])
            nc.sync.dma_start(out=wq_sb,
                              in_=wq[:].rearrange("(j p) m -> p j m", p=128))
            nc.sync.dma_start(out=bq_sb, in_=bqv[:])
            # Pool engine (SBUF-only work): keeps the DVE stream head clear
            # for the LN stats that gate the first exp
            nc.gpsimd.tensor_copy(dyn_bf, dyn_sb)
            for t in range(NT):
                for h in range(3):
                    nc.gpsimd.memset(v_sb[t][:, 65 * h + 64:65 * h + 65], 1.0)

            # ---- phase 1/2 emitters ----
            def emit_stats(st):
                t_ = hs[:, st, :]
                stt = stats.tile([128, 3, 6], F32, tag="st", name=f"st{st}")
                for sg in range(3):
                    nc.vector.bn_stats(out=stt[:, sg, :],
                                       in_=t_[:, sg * 256:(sg + 1) * 256])
                nc.vector.bn_aggr(out=mv_all[:, :, st], in_=stt)

            def emit_newton(c):
                # rstd = 1/sqrt(var+eps) on DVE via Newton (hidden_states is
                # ~N(0,1) so var+eps stays near 1 and y0=1 converges in 3
                # steps to ~1e-6) — keeps Sqrt (and its activation-table
                # load) off the Act engine, whose exp stream is the
                # critical resource.
                # runs on Pool: it is idle this early, so the 10-op serial
                # chain isn't stretched by greedy backfill the way it would
                # be between DVE stats ops
                sl = slice(4 * c, 4 * (c + 1))
                x = stats.tile([128, 4], F32, tag="nx", name=f"nx{c}")
                y = rstd_all[:, sl]
                nc.gpsimd.tensor_scalar(out=x, in0=mv_all[:, 1, sl],
                                        scalar1=LN_EPS, scalar2=None,
                                        op0=mybir.AluOpType.add)
                nc.gpsimd.tensor_scalar(out=y, in0=x, scalar1=-0.5,
                                        scalar2=1.5,
                                        op0=mybir.AluOpType.mult,
                                        op1=mybir.AluOpType.add)
                for it in range(2):
                    t2 = stats.tile([128, 4], F32, tag="nt", name=f"nt{c}{it}")
                    nc.gpsimd.tensor_tensor(t2, y, y, mybir.AluOpType.mult)
                    nc.gpsimd.tensor_tensor(t2, t2, x, mybir.AluOpType.mult)
                    nc.gpsimd.tensor_scalar(out=t2, in0=t2, scalar1=-0.5,
                                            scalar2=1.5,
                                            op0=mybir.AluOpType.mult,
                                            op1=mybir.AluOpType.add)
                    nc.gpsimd.tensor_tensor(y, y, t2, mybir.AluOpType.mult)

            def emit_norm(st):
                t_ = hs[:, st, :]
                nc.vector.tensor_scalar(out=t_, in0=t_,
                                        scalar1=mv_all[:, 0, st:st + 1],
                                        scalar2=rstd_all[:, st:st + 1],
                                        op0=mybir.AluOpType.subtract,
                                        op1=mybir.AluOpType.mult)
                nc.sync.dma_start_transpose(
                    xT[:, :, st * 128:(st + 1) * 128], t_)

            # Projection chains run in 256-wide chunks through a 4-deep pool
            # of 1-bank PSUM tiles, and their PSUM drains go to the Pool
            # engine — DVE is saturated with LN stats exactly when these
            # need to retire, and the pool rotation would otherwise chain
            # every projection to a stalled DVE copy.
            def emit_kproj(c, mi):
                m0_, msz = MT[mi]
                ps = mm512.tile([128, 512], F32, tag="mm", name=f"psk{c}{mi}")
                for j in range(6):
                    nc.tensor.matmul(ps[:msz], wk_sb[:, j, m0_:m0_ + msz],
                                     crsT[:, j, c * 512:(c + 1) * 512],
                                     start=(j == 0), stop=(j == 5))
                src = (m0T0[:, c * 512:(c + 1) * 512] if mi == 0
                       else m0T1[:, c * 512:(c + 1) * 512])
                nc.vector.tensor_tensor(kT[mi][:, c * 512:(c + 1) * 512],
                                        ps[:msz], src, mybir.AluOpType.add)

            def emit_qproj(c, mi):
                m0_, msz = MT[mi]
                ps = mm512.tile([128, 512], F32, tag="mm", name=f"psq{c}{mi}")
                for j in range(6):
                    nc.tensor.matmul(ps[:msz], wq_sb[:, j, m0_:m0_ + msz],
                                     xT[:, j, c * 512:(c + 1) * 512],
                                     start=(j == 0), stop=(j == 5))
                nc.vector.tensor_scalar(
                    out=qT[mi][:, c * 512:(c + 1) * 512],
                    in0=ps[:msz], scalar1=bq_sb[:msz, mi:mi + 1],
                    scalar2=None, op0=mybir.AluOpType.add)

            def emit_vproj(st):
                ps = mm512.tile([128, 512], F32, tag="mm", name=f"psv{st}")
                pv = ps[:, 0:HL]
                for j in range(6):
                    nc.tensor.matmul(pv, crsT[:, j, st * 128:(st + 1) * 128],
                                     wv_sb[:, j, :],
                                     start=(j == 0), stop=(j == 5))
                nc.vector.tensor_tensor(
                    v_sb[st].rearrange("p (h m) -> p h m", m=65)[:, :, 0:64],
                    pv.rearrange("p (h m) -> p h m", m=64),
                    m1_sb[:, st, :].rearrange("p (h m) -> p h m", m=64),
                    mybir.AluOpType.add)

            # ---- phase A: ONLY the first-exp critical chain at high
            # priority. The Tile scheduler is greedy by (ready, emission
            # priority), so everything emitted later still hoists into idle
            # gaps automatically — emission position is a deadline, not a
            # start time.
            for st in range(4):
                emit_stats(st)
            emit_newton(0)
            for st in range(4):
                emit_norm(st)
            for st in range(4, 8):
                emit_stats(st)
            emit_newton(1)
            for st in range(4, 8):
                emit_norm(st)
            # crsT chunks 1-3: must be emitted before the kproj consumers
            # below; half-chunk granularity so a bulk transfer never blocks
            # a just-became-ready xT transpose for long.
            def dma_crsT_half(c, v):
                nc.sync.dma_start(
                    out=crsT[:, :, c * 512 + v * 256:c * 512 + (v + 1) * 256],
                    in_=crsT_d[:, c * 512 + v * 256:c * 512 + (v + 1) * 256]
                    .rearrange("(j p) s -> p j s", p=128))

            for c in (1, 2, 3):
                dma_crsT_half(c, 0)
                dma_crsT_half(c, 1)

            # creation order drives the PSUM pool rotation: kproj(0,0) is
            # ready first, so it must own the first buffer. All m-tile-0
            # k projections fit in the pre-exp PE idle window, so no score
            # ever waits on kT.
            emit_kproj(0, 0)
            emit_qproj(0, 0)
            emit_qproj(1, 0)
            # kproj(1..3, 0) moved into step-0's filler queue: emitted after
            # the first score+exp, so the exp's batched PE-sem tick does not
            # cover them (they retire late, gated on the crsT chunk DMAs,
            # and would otherwise hold the first exp hostage).

            # Late input group: lower priority than everything above.
            nc.sync.dma_start(out=wv_sb,
                              in_=wv[:].rearrange("(j p) m -> p j m", p=128))
            nc.sync.dma_start(
                out=m1_sb, in_=m1v[:].rearrange("(c p) m -> p c m", p=128))
            for u in range(4, 8):
                dma_hs2(u)
            nc.sync.dma_start(out=m0T1, in_=m0T_d[128:192, :])
            nc.sync.dma_start(out=wo_sb0, in_=wo[0:128, :])
            nc.sync.dma_start(out=wo_sb1, in_=wo[128:193, :])

            # ---- phase 3: attention steps; bulk work is emitted at its
            # deadline position inside the exp-paced loop.
            steps = [(n, h) for n in range(NCH) for h in range(3)]

            def emit_ln_tail(c):
                for st in range(4 * c, 4 * (c + 1)):
                    emit_stats(st)
                emit_newton(c)
                for st in range(4 * c, 4 * (c + 1)):
                    emit_norm(st)

            fillers = deque()
            late_fillers = deque()
            step_fillers = {
                0: [lambda: emit_kproj(1, 0), lambda: emit_kproj(2, 0),
                    lambda: emit_kproj(3, 0)] +
                   [lambda t=t: emit_vproj(t) for t in range(NT)],
                1: [lambda: emit_kproj(0, 1), lambda: emit_kproj(1, 1),
                    lambda: emit_kproj(2, 1), lambda: emit_kproj(3, 1),
                    lambda: emit_qproj(0, 1), lambda: emit_qproj(1, 1),
                    lambda: emit_ln_tail(2)],
                2: [lambda: emit_ln_tail(3),
                    lambda: emit_qproj(2, 0), lambda: emit_qproj(3, 0),
                    lambda: emit_qproj(2, 1), lambda: emit_qproj(3, 1)],
                3: [], 4: [], 5: [],
            }

            a_tiles = {}
            # 8 attn-out accumulators packed into one 2-bank PSUM tile; qb=7
            # starts at the second bank so no slice straddles a boundary.
            av_big = avp.tile([128, 1024], F32, tag="av", name="av_big")
            av_tiles = [av_big[:, qb * 65:qb * 65 + 65] if qb < 7
                        else av_big[:, 512:577] for qb in range(NQB)]


            def head_rows(h, tens):
                return tens[0][64 * h:64 * h + 64, :] if h < 2 else \
                    tens[1][0:64, :]

            def emit_av_zero():
                # The 8 packed accumulators share PSUM zero-regions, so
                # matmul start=True zeroing is poison (each start re-marks
                # the whole 2KB region pending-zero, wiping its neighbours'
                # partial sums). Zero explicitly and accumulate-only.
                nc.vector.memset(av_big[:, 0:7 * 65], 0.0)
                nc.vector.memset(av_big[:, 512:577], 0.0)

            def emit_attnv_t(k_idx, t, qb0=0, qb1=NQB):
                n, h = steps[k_idx]
                et = get_et(k_idx, t)
                for qb in range(qb0, qb1):
                    nc.tensor.matmul(av_tiles[qb],
                                     et[:, qb * 128:(qb + 1) * 128],
                                     v_sb[t][:, 65 * h:65 * h + 65],
                                     start=False, stop=(t == NT - 1),
                                     skip_group_check=True)

            def emit_scale_qb(k_idx, qb):
                n, h = steps[k_idx]
                st = n * NQB + qb
                if debug and k_idx == 0 and qb == 0:
                    dav = srec.tile([128, 512], F32, tag="dav", name="dav",
                                    bufs=1)
                    nc.vector.tensor_copy(dav[:, 0:455], av_big[:, 0:455])
                    nc.sync.dma_start(out=dbg_av[:, 0:455], in_=dav[:, 0:455])
                    nc.vector.tensor_copy(dav[:, 0:65], av_big[:, 512:577])
                    nc.sync.dma_start(out=dbg_av[:, 512:577], in_=dav[:, 0:65])
                if (n, qb) not in a_tiles:
                    a_tiles[(n, qb)] = ap_.tile([128, 256], BF16,
                                                tag=f"a{qb}", bufs=2,
                                                name=f"a{qb}_{n}")
                at = a_tiles[(n, qb)]
                av = av_tiles[qb]
                r = srec.tile([128, 1], F32, tag="r", name=f"r{k_idx}{qb}")
                nc.vector.reciprocal(out=r, in_=av[:, 64:65])
                nc.vector.tensor_tensor(r, r, dyn_sb[:, st:st + 1],
                                        mybir.AluOpType.mult)
                nc.vector.tensor_scalar(out=at[:, 64 * h:64 * h + 64],
                                        in0=av[:, 0:64], scalar1=r,
                                        scalar2=None,
                                        op0=mybir.AluOpType.mult)
                if h == 2:
                    nc.vector.tensor_copy(
                        at[:, 192:256],
                        dyn_bf[:, st:st + 1].to_broadcast((128, 64)))
                    nc.sync.dma_start_transpose(
                        catB[:, :, st * 128:(st + 1) * 128], at[:])
                    del a_tiles[(n, qb)]

            def emit_scales(k_idx):
                for qb in range(NQB):
                    emit_scale_qb(k_idx, qb)
                # re-zero for the next step's accumulate-only attn-v
                emit_av_zero()

            def emit_outproj(st):
                ot = srec.tile([128, H], F32, tag="ot", bufs=6,
                               name=f"ot{st}")
                for hi, n0 in enumerate((0, 384)):
                    wp = mm512.tile([128, 512], F32, tag="mm",
                                    name=f"wp{st}_{n0}")
                    nc.tensor.matmul(wp[:, 0:384],
                                     catB[:, 0, st * 128:(st + 1) * 128],
                                     wo_sb0[:, n0:n0 + 384],
                                     start=True, stop=False)
                    nc.tensor.matmul(wp[:, 0:384],
                                     catB[0:65, 1, st * 128:(st + 1) * 128],
                                     wo_sb1[:, n0:n0 + 384],
                                     start=False, stop=True)
                    # chunk-0 copies go Pool-only: DVE must stay clear for
                    # the softmax scales (an ot-copy stuck in the DVE stream
                    # head-of-line-blocks them and stalls the whole cat/
                    # outproj pipeline). The drain chunk has no scales left,
                    # so it splits across both engines.
                    if st < NQB + 4 or hi == 0:
                        nc.vector.tensor_copy(ot[:, n0:n0 + 384], wp[:, 0:384])
                    else:
                        # drain-chunk second halves on the (by then idle) Act
                        nc.scalar.activation(
                            out=ot[:, n0:n0 + 384], in_=wp[:, 0:384],
                            func=mybir.ActivationFunctionType.Copy, bias=0.0)
                nc.sync.dma_start(
                    out=out[st * 128:(st + 1) * 128, :], in_=ot)

            et_tiles = {}

            def get_et(k, t):
                d = et_tiles.setdefault(k, {})
                if t not in d:
                    d[t] = etp.tile([128, CW], BF16, tag=f"e{t}", bufs=1,
                                    name=f"e{t}_{k}")
                return d[t]

            def emit_pretile(nk):
                # Next step's first score tile, computed into the (idle at
                # step tail) mm512 pool + exp'd ahead of the boundary: the
                # Act stream rolls straight into step nk without waiting
                # for the sps buffer rotation to free a score tile.
                nn2, nh2 = steps[nk]
                kk2 = head_rows(nh2, kT)
                qq2 = head_rows(nh2, qT)
                w = 512 if nk == len(steps) - 1 else CW
                for v2 in range(w // 512):
                    psb = mm512.tile([128, 512], F32, tag="mm",
                                     name=f"pre{nk}{v2}")
                    c0 = nn2 * CW + v2 * 512
                    nc.tensor.matmul(psb, kk2[:, 0:128], qq2[:, c0:c0 + 512],
                                     start=True, stop=True)
                    nc.scalar.activation(
                        out=get_et(nk, 0)[:, v2 * 512:(v2 + 1) * 512],
                        in_=psb,
                        func=mybir.ActivationFunctionType.Exp, scale=1.0)

            for k_idx, (n, h) in enumerate(steps):
                kk = head_rows(h, kT)
                qq = head_rows(h, qT)
                fillers.extend(step_fillers[k_idx])
                if k_idx == 3:
                    late_fillers.extend(
                        [lambda st=st: emit_outproj(st) for st in range(0, 4)])
                elif k_idx == 4:
                    late_fillers.extend(
                        [lambda st=st: emit_outproj(st) for st in range(4, NQB)])
                for t in range(NT):
                    get_et(k_idx, t)
                last = k_idx == len(steps) - 1
                first = k_idx == 0
                if first:
                    emit_av_zero()
                # The final step runs as two 512-wide half-chunks so the
                # first half's softmax scales / cat transpose / output
                # projection overlap the second half's exps instead of all
                # landing in the post-Act drain.
                halves = ((0, 512), (1, 512)) if last else ((0, CW),)
                for u, uw in halves:
                    for t in range(NT):
                        sp = sps.tile([128, CW], F32, tag="sc",
                                      name=f"sp{k_idx}{u}{t}")
                        # the very first score tile runs as two 512-wide
                        # halves so the first exp gates only on qproj(0,0),
                        # not on the whole first q-chunk
                        splitx = first and t == 0
                        for v2 in range(uw // 512):
                            c0 = n * CW + u * 512 + v2 * 512
                            nc.tensor.matmul(
                                sp[:, v2 * 512:(v2 + 1) * 512],
                                kk[:, t * 128:(t + 1) * 128],
                                qq[:, c0:c0 + 512],
                                start=True, stop=True)
                            if splitx:
                                nc.scalar.activation(
                                    out=get_et(k_idx, t)[
                                        :, v2 * 512:(v2 + 1) * 512],
                                    in_=sp[:, v2 * 512:(v2 + 1) * 512],
                                    func=mybir.ActivationFunctionType.Exp,
                                    scale=1.0)
                        if not splitx:
                            nc.scalar.activation(
                                out=get_et(k_idx, t)[:, u * 512:u * 512 + uw],
                                in_=sp[:, 0:uw],
                                func=mybir.ActivationFunctionType.Exp,
                                scale=1.0)
                        if debug and k_idx == 0 and t == 0 and u == 0:
                            nc.sync.dma_start(out=dbg_et[:],
                                              in_=get_et(0, 0)[:])
                        # drain filler emissions fast enough that producers
                        # (e.g. v projections) are always emitted before
                        # their consumers; the scheduler floats them into
                        # whatever idle slots exist.
                        for _ in range(2 if len(fillers) > 10 else 1):
                            if fillers:
                                fillers.popleft()()
                        if t >= 8 and late_fillers:
                            late_fillers.popleft()()
                        if last:
                            # u0's softmax scales interleave into u1's loop:
                            # emitting them between the halves would give
                            # them earlier per-engine sem ticks than u1's
                            # scores, and the monotone tick waits would then
                            # serialize u1's whole stream behind the u0
                            # scale/cat chain (~5us Act gap).
                            if u == 1 and t < 8 and t % 2 == 0:
                                emit_scale_qb(k_idx, t // 2)
                            if t >= 8:
                                emit_attnv_t(k_idx, t - 8, u * 4, u * 4 + 4)
                        elif t >= 10:
                            # emission lag keeps these from
                            # head-of-line-blocking the next scores while
                            # the previous step's scales still own av
                            emit_attnv_t(k_idx, t - 10)
                    if last:
                        for t_ in range(NT - 8, NT):
                            emit_attnv_t(k_idx, t_, u * 4, u * 4 + 4)
                        if u == 0:
                            late_fillers.extend(
                                [lambda st=st: emit_outproj(st)
                                 for st in range(n * NQB, n * NQB + 4)])
                        else:
                            for qb in range(4, 8):
                                emit_scale_qb(k_idx, qb)
                            for st in range(n * NQB + 4, n * NQB + 8):
                                emit_outproj(st)
                if not last:
                    for t in range(NT - 10, NT):
                        emit_attnv_t(k_idx, t)
                    def s2(k, q0):
                        emit_scale_qb(k, q0)
                        emit_scale_qb(k, q0 + 1)
                        if q0 == 6:
                            # re-zero for the next step's accumulate-only
                            # attn-v; popped well before its lag-7 emission
                            emit_av_zero()
                    for q0 in (0, 2, 4, 6):
                        fillers.append(lambda k=k_idx, q=q0: s2(k, q))

            # ---- drain ----
            while fillers:
                fillers.popleft()()
            while late_fillers:
                late_fillers.popleft()()

            if debug:
                nc.sync.dma_start(out=dbg_qT[:], in_=qT[0][:])
                nc.sync.dma_start(out=dbg_kT[:], in_=kT[0][:])
                nc.sync.dma_start(out=dbg_v[:], in_=v_sb[0][:])
                nc.sync.dma_start(out=dbg_cat[:], in_=catB[:])
                nc.sync.dma_start(out=dbg_x[:], in_=xT[:])

    nc.compile()
    return nc


def make_in_maps(inputs):
    bf = lambda a: np.asarray(np.asarray(a, np.float32), BF16_NP)
    hs = np.asarray(inputs["hidden_states"], np.float32)
    cs = np.asarray(inputs["cross_states"], np.float32)
    mem = np.asarray(inputs["memory_tensors"], np.float32)
    dyn = np.asarray(inputs["dynamic_factor"], np.float32)
    Wq = np.asarray(inputs["Wq"], np.float32)
    Wk = np.asarray(inputs["Wk"], np.float32)
    Wv = np.asarray(inputs["Wv"], np.float32)
    Wo = np.asarray(inputs["Wo"], np.float32)
    bq = np.asarray(inputs["bq"], np.float32)
    bv = np.asarray(inputs["bv"], np.float32)
    bo = np.asarray(inputs["bo"], np.float32)
    gate = float(np.asarray(inputs["gate"]).reshape(-1)[0])
    gate_bias = float(np.asarray(inputs["gate_bias"]).reshape(-1)[0])
    ln_g = np.asarray(inputs["ln_g"], np.float32)
    ln_b = np.asarray(inputs["ln_b"], np.float32)

    isq = 1.0 / np.sqrt(HD)
    in_maps = []
    for core in range(8):
        b, g = divmod(core, NG)
        cols = slice(g * HL, (g + 1) * HL)
        wq_eff = ln_g[:, None] * Wq[:, cols] * isq
        bq_eff = (bq[cols] + ln_b @ Wq[:, cols]) * isq
        bq_pack = np.zeros((128, 2), np.float32)
        bq_pack[:, 0] = bq_eff[0:128]
        bq_pack[:64, 1] = bq_eff[128:192]
        wo_ext = np.zeros((HL + 1, H), np.float32)
        wo_ext[:HL] = Wo[cols, :] * gate
        if g == 0:
            wo_ext[HL] = bo * gate + gate_bias
        in_maps.append({
            "hid": bf(hs[b]),
            "crsT": bf(np.ascontiguousarray(cs[b].T)),
            "m0T": bf(np.ascontiguousarray((mem[0, b][:, cols] * MEM_W).T)),
            "m1v": bf(mem[1, b][:, cols] * MEM_W + bv[cols]),
            "wq": bf(wq_eff),
            "wk": bf(Wk[:, cols]),
            "wv": bf(Wv[:, cols]),
            "wo": bf(wo_ext),
            "bqv": np.ascontiguousarray(bq_pack),
            "dynv": np.ascontiguousarray(dyn[b, :, 0]),
        })
    return in_maps


def kernel(**inputs):
    mask = np.asarray(inputs["attention_mask"])
    if not np.all(mask != 0):
        raise NotImplementedError("kernel specialized for all-ones attention_mask")

    if "nc" not in _CACHED:
        _CACHED["nc"] = build_bass()
    nc = _CACHED["nc"]

    from concourse.bass_utils import run_bass_kernel_spmd
    in_maps = make_in_maps(inputs)
    trace = bool(int(os.environ.get("KERNEL_TRACE", "0")))
    r = run_bass_kernel_spmd(nc, in_maps, list(range(8)), trace=trace)
    _CACHED["exec_time_ns"] = r.exec_time_ns
    _CACHED["profile_json"] = r.profile_json
    _CACHED["trace"] = r.instructions_and_trace
    res = r.results

    out = np.zeros((B, S, H), np.float32)
    for core in range(8):
        b = core // NG
        out[b] += res[core]["out"]
    return out
